# revision 5
# baseline (speedup 1.0000x reference)
"""Supervised contrastive loss on 8 Trainium2 NeuronCores — symmetric scheme.

Reference computation (N=8192, D=128, TAU=0.1, 100 classes):
    xn   = x / ||x||_row
    sim  = xn @ xn.T                      [N, N]
    e    = exp(sim / TAU)
    top  = sum_j e[i,j] * (y_i == y_j)
    down = sum_j e[i,j]
    loss = mean(log(down) - log(top))

e is symmetric, so only ~62.5% of the N^2 area is computed. The matrix is
tiled into 64 i-tiles (128 rows) x 4 column groups G (2048 cols). Per
(core, G): [0,2,4,6][G] "full" jobs (i-tiles strictly above G's diagonal
square, dealt round-robin by rho%8) plus 2 diagonal-square jobs -> 20
uniform jobs per core, identical program on every core (per-core data via
inputs only).

Per job: bf16 GEMM -> PSUM [128,2048]; exp on ACT in two 1024-halves
(accum_out = down row-sums); stt on DVE (masked top row-sums). Full jobs
additionally stream e through a class-colsum matmul (one-hot + ones lhsT)
accumulated in a per-G PSUM [128, 2048]; that per-class column-sum block
is copied out and the host gathers top-col = cls[y_j, j], down-col =
cls[100, j] for the mirror (lower-triangle) contributions.

Host does normalization/transpose/bf16-cast of x and the final log/mean.
"""

import sys

import numpy as np

sys.path.insert(0, "/opt/trn_rl_repo")

import ml_dtypes

TAU = 0.1
N, D = 8192, 128
P = 128
NCORES = 8
CH = 2048                   # column group width (4 PSUM banks)
NG = 4                      # column groups
NJOBS = 20                  # jobs per core (uniform)
NFULL = 12                  # full (cls-contributing) jobs per core
MM_N = 512                  # one PSUM bank of f32 per matmul

_PROGRAM = None


def job_table(core):
    """Uniform job list: (rho, G, kind); identical structure for all cores."""
    jobs = []
    for G in range(4):
        for r in [r for r in range(16 * G) if r % 8 == core]:
            jobs.append((r, G, "full"))
        for r in (16 * G + 2 * core, 16 * G + 2 * core + 1):
            jobs.append((r, G, "diag"))
    return jobs


# job structure shared by every core: kinds/G per jid
_JOBS0 = job_table(0)
assert len(_JOBS0) == NJOBS
assert sum(1 for _, _, k in _JOBS0 if k == "full") == NFULL


def _build_program():
    import concourse.bacc as bacc
    import concourse.bass as bass  # noqa: F401
    import concourse.mybir as mybir
    from concourse.tile import TileContext

    f32 = mybir.dt.float32
    bf16 = mybir.dt.bfloat16
    AF = mybir.ActivationFunctionType
    OP = mybir.AluOpType

    nc = bacc.Bacc("TRN2", target_bir_lowering=False)
    xnT_h = nc.declare_dram_parameter("xnT", [P, N], bf16, isOutput=False)
    xiT_h = nc.declare_dram_parameter("xiT", [P, NJOBS * P], bf16, isOutput=False)
    ycl_h = nc.declare_dram_parameter("ycls", [P, NFULL * P], bf16, isOutput=False)
    yb_h = nc.declare_dram_parameter("y_bcast", [P, N], bf16, isOutput=False)
    yi_h = nc.declare_dram_parameter("y_own", [P, NJOBS], f32, isOutput=False)
    out_h = nc.declare_dram_parameter("out", [P, 3 * NJOBS], f32, isOutput=True)
    cls_h = nc.declare_dram_parameter("cls_out", [P, 3 * CH], bf16, isOutput=True)

    with TileContext(nc) as tc:
        with tc.tile_pool(name="persist", bufs=1) as persist:
            xnT = persist.tile([P, N], bf16)
            xiT = persist.tile([P, NJOBS * P], bf16)
            ycl = persist.tile([P, NFULL * P], bf16)
            ybc = persist.tile([P, N], bf16)
            yis = persist.tile([P, NJOBS], f32)
            outs = persist.tile([P, 3 * NJOBS], f32)

            nc.sync.dma_start(out=yis[:], in_=yi_h[:, :])
            nc.sync.dma_start(out=xiT[:], in_=xiT_h[:, :])
            nc.gpsimd.dma_start(out=ycl[:], in_=ycl_h[:, :])
            for g in range(NG):
                nc.sync.dma_start(
                    out=xnT[:, g * CH : (g + 1) * CH],
                    in_=xnT_h[:, g * CH : (g + 1) * CH],
                )
                nc.gpsimd.dma_start(
                    out=ybc[:, g * CH : (g + 1) * CH],
                    in_=yb_h[:, g * CH : (g + 1) * CH],
                )

            with (
                tc.tile_pool(name="mpsum", bufs=1, space="PSUM") as mpp,
                tc.tile_pool(name="clsp", bufs=1, space="PSUM") as clp,
                tc.tile_pool(name="ep", bufs=3) as ep,
                tc.tile_pool(name="csb", bufs=2) as csp,
                tc.tile_pool(name="trashp", bufs=1) as trp,
            ):
                trash = trp.tile([P, CH], bf16)
                jid = 0
                cid = 0
                for G in range(NG):
                    nfull = [0, 2, 4, 6][G]
                    njobs = nfull + 2
                    cls = None
                    if nfull:
                        cls = clp.tile([P, CH], f32, tag="cls", name=f"cls{G}")
                    for s in range(njobs):
                        ps = mpp.tile([P, CH], f32, tag="ps", name=f"ps{jid}")
                        for k in range(CH // MM_N):
                            nc.tensor.matmul(
                                out=ps[:, k * MM_N : (k + 1) * MM_N],
                                lhsT=xiT[:, jid * P : (jid + 1) * P],
                                rhs=xnT[:, G * CH + k * MM_N : G * CH + (k + 1) * MM_N],
                                start=True,
                                stop=True,
                            )
                        e = ep.tile([P, CH], bf16, tag="e", name=f"e{jid}")
                        for h in range(2):
                            hw = CH // 2
                            nc.scalar.activation(
                                out=e[:, h * hw : (h + 1) * hw],
                                in_=ps[:, h * hw : (h + 1) * hw],
                                func=AF.Exp,
                                scale=1.0 / TAU,
                                accum_out=outs[:, NJOBS + 2 * jid + h :
                                                NJOBS + 2 * jid + h + 1],
                            )
                        nc.vector.scalar_tensor_tensor(
                            out=trash[:],
                            in0=ybc[:, G * CH : (G + 1) * CH],
                            scalar=yis[:, jid : jid + 1],
                            in1=e[:],
                            op0=OP.is_equal,
                            op1=OP.mult,
                            accum_out=outs[:, jid : jid + 1],
                        )
                        if s < nfull:
                            for k in range(CH // MM_N):
                                nc.tensor.matmul(
                                    out=cls[:, k * MM_N : (k + 1) * MM_N],
                                    lhsT=ycl[:, cid * P : (cid + 1) * P],
                                    rhs=e[:, k * MM_N : (k + 1) * MM_N],
                                    start=(s == 0),
                                    stop=(s == nfull - 1),
                                    skip_group_check=True,
                                )
                            cid += 1
                        jid += 1
                    if nfull:
                        # copy cls psum -> sbuf bf16 (split across DVE/ACT),
                        # then stream to DRAM; host does the column gather.
                        clsS = csp.tile([P, CH], bf16, tag="clsS", name=f"clsS{G}")
                        nc.vector.tensor_copy(
                            out=clsS[:, : CH // 2], in_=cls[:, : CH // 2]
                        )
                        nc.scalar.copy(
                            out=clsS[:, CH // 2 :], in_=cls[:, CH // 2 :]
                        )
                        nc.sync.dma_start(
                            out=cls_h[:, (G - 1) * CH : G * CH], in_=clsS[:]
                        )
                assert jid == NJOBS and cid == NFULL
            nc.gpsimd.dma_start(out=out_h[:, :], in_=outs[:])
    nc.compile()
    return nc


def _get_program():
    global _PROGRAM
    if _PROGRAM is None:
        _PROGRAM = _build_program()
    return _PROGRAM


def make_in_maps(x, y):
    x = np.asarray(x, dtype=np.float64)
    yf = np.asarray(y).astype(np.float32)
    xn = x / np.linalg.norm(x, axis=-1, keepdims=True)
    xnT = np.ascontiguousarray(xn.T.astype(ml_dtypes.bfloat16))   # [D, N]
    ybc = np.ascontiguousarray(
        np.broadcast_to(yf.astype(ml_dtypes.bfloat16)[None, :], (P, N))
    )
    yi = np.asarray(y).astype(np.int64)
    in_maps = []
    for core in range(NCORES):
        jobs = job_table(core)
        xiT = np.empty((P, NJOBS * P), ml_dtypes.bfloat16)
        yis = np.empty((P, NJOBS), np.float32)
        ycl = np.zeros((P, NFULL * P), ml_dtypes.bfloat16)
        cid = 0
        for jid, (rho, G, kind) in enumerate(jobs):
            rows = slice(rho * P, (rho + 1) * P)
            xiT[:, jid * P : (jid + 1) * P] = xnT[:, rows]
            yis[:, jid] = yf[rows]
            if kind == "full":
                Y = np.zeros((P, P), np.float32)
                Y[np.arange(P), yi[rows]] = 1.0
                Y[:, 100] = 1.0
                ycl[:, cid * P : (cid + 1) * P] = Y.astype(ml_dtypes.bfloat16)
                cid += 1
        in_maps.append(
            {
                "xnT": xnT,
                "xiT": np.ascontiguousarray(xiT),
                "ycls": np.ascontiguousarray(ycl),
                "y_bcast": ybc,
                "y_own": yis,
            }
        )
    return in_maps


def finalize(per_core_outs, per_core_cls, y):
    yi = np.asarray(y).astype(np.int64)
    down = np.zeros(N, np.float64)
    top = np.zeros(N, np.float64)
    for core in range(NCORES):
        o = np.asarray(per_core_outs[core], dtype=np.float64)  # [P, 3*NJOBS]
        cl = np.asarray(per_core_cls[core], dtype=np.float64)  # [P, 3*CH]
        for jid, (rho, G, kind) in enumerate(job_table(core)):
            rows = slice(rho * P, (rho + 1) * P)
            top[rows] += o[:, jid]
            down[rows] += o[:, NJOBS + 2 * jid] + o[:, NJOBS + 2 * jid + 1]
        for G in range(1, 4):
            blk = cl[:, (G - 1) * CH : G * CH]                 # [128 c', 2048 j]
            cols = np.arange(G * CH, (G + 1) * CH)
            down[cols] += blk[100, :]
            top[cols] += blk[yi[cols], np.arange(CH)]
    return np.float32(np.mean(np.log(down) - np.log(top)))


def kernel(x, y):
    from concourse.bass_utils import run_bass_kernel_spmd

    nc = _get_program()
    in_maps = make_in_maps(x, y)
    res = run_bass_kernel_spmd(nc, in_maps, list(range(NCORES)))
    return finalize(
        [r["out"] for r in res.results],
        [r["cls_out"] for r in res.results],
        y,
    )


# revision 12
# speedup vs baseline: 1.5709x; 1.5709x over previous
"""Supervised contrastive loss on 8 Trainium2 NeuronCores — symmetric scheme.

Reference computation (N=8192, D=128, TAU=0.1, 100 classes):
    xn   = x / ||x||_row
    sim  = xn @ xn.T                      [N, N]
    e    = exp(sim / TAU)
    top  = sum_j e[i,j] * (y_i == y_j)
    down = sum_j e[i,j]
    loss = mean(log(down) - log(top))

e is symmetric, so only ~62.5% of the N^2 area is computed. The matrix is
tiled into 64 i-tiles (128 rows) x 4 column groups G (2048 cols). Per
(core, G): [0,2,4,6][G] "full" jobs (i-tiles strictly above G's diagonal
square, dealt round-robin by rho%8) plus 2 diagonal-square jobs -> 20
uniform jobs per core, identical program on every core (per-core data via
inputs only).

Per job: bf16 GEMM -> PSUM [128,2048]; exp on ACT in two 1024-halves
(accum_out = down row-sums); stt on DVE (masked top row-sums). Full jobs
additionally stream e through a class-colsum matmul (one-hot + ones lhsT)
accumulated in a per-G PSUM [128, 2048]; that per-class column-sum block
is copied out and the host gathers top-col = cls[y_j, j], down-col =
cls[100, j] for the mirror (lower-triangle) contributions.

Host does normalization/transpose/bf16-cast of x and the final log/mean.
"""

import sys

import numpy as np

sys.path.insert(0, "/opt/trn_rl_repo")

import ml_dtypes

TAU = 0.1
N, D = 8192, 128
P = 128
NCORES = 8
CH = 2048                   # column group width (4 PSUM banks)
NG = 4                      # column groups
NJOBS = 20                  # jobs per core (uniform)
NFULL = 12                  # full (cls-contributing) jobs per core
MM_N = 512                  # one PSUM bank of f32 per matmul

_PROGRAM = None


def job_table(core):
    """Uniform job list: (rho, G, kind); identical structure for all cores."""
    jobs = []
    for G in range(4):
        for r in [r for r in range(16 * G) if r % 8 == core]:
            jobs.append((r, G, "full"))
        for r in (16 * G + 2 * core, 16 * G + 2 * core + 1):
            jobs.append((r, G, "diag"))
    return jobs


# job structure shared by every core: kinds/G per jid
_JOBS0 = job_table(0)
assert len(_JOBS0) == NJOBS
assert sum(1 for _, _, k in _JOBS0 if k == "full") == NFULL


def _build_program():
    import concourse.bacc as bacc
    import concourse.bass as bass  # noqa: F401
    import concourse.mybir as mybir
    from concourse.tile import TileContext

    f32 = mybir.dt.float32
    bf16 = mybir.dt.bfloat16
    AF = mybir.ActivationFunctionType
    OP = mybir.AluOpType

    nc = bacc.Bacc("TRN2", target_bir_lowering=False)
    xnT_h = nc.declare_dram_parameter("xnT", [P, N], bf16, isOutput=False)
    xiT_h = nc.declare_dram_parameter("xiT", [P, NJOBS * P], bf16, isOutput=False)
    ycl_h = nc.declare_dram_parameter("ycls", [P, NFULL * P], bf16, isOutput=False)
    yb_h = nc.declare_dram_parameter("y_bcast", [P, N], bf16, isOutput=False)
    yi_h = nc.declare_dram_parameter("y_own", [P, NJOBS], f32, isOutput=False)
    out_h = nc.declare_dram_parameter("out", [P, 3 * NJOBS], f32, isOutput=True)
    cls_h = nc.declare_dram_parameter("cls_out", [P, 3 * CH], bf16, isOutput=True)

    with TileContext(nc) as tc:
        with tc.tile_pool(name="persist", bufs=1) as persist:
            xnT = persist.tile([P, N], bf16)
            xiT = persist.tile([P, NJOBS * P], bf16)
            ycl = persist.tile([P, NFULL * P], bf16)
            ybc = persist.tile([P, N], bf16)
            yis = persist.tile([P, NJOBS], f32)
            outs = persist.tile([P, 3 * NJOBS], f32)

            nc.sync.dma_start(out=yis[:], in_=yi_h[:, :])
            nc.sync.dma_start(out=xiT[:], in_=xiT_h[:, :])
            nc.gpsimd.dma_start(out=ycl[:], in_=ycl_h[:, :])
            for g in range(NG):
                nc.sync.dma_start(
                    out=xnT[:, g * CH : (g + 1) * CH],
                    in_=xnT_h[:, g * CH : (g + 1) * CH],
                )
                nc.gpsimd.dma_start(
                    out=ybc[:, g * CH : (g + 1) * CH],
                    in_=yb_h[:, g * CH : (g + 1) * CH],
                )

            with (
                tc.tile_pool(name="mpsum", bufs=2, space="PSUM") as mpp,
                tc.tile_pool(name="clsp", bufs=1, space="PSUM") as clp,
                tc.tile_pool(name="ep", bufs=3) as ep,
                tc.tile_pool(name="csb", bufs=2) as csp,
                tc.tile_pool(name="trashp", bufs=1) as trp,
            ):
                trash = trp.tile([P, CH], bf16)

                def flush_cls(item):
                    # cls matmuls delayed one job so the PE never waits on
                    # the exp of the job it just multiplied
                    cls_t, e_t, _cid, is_start, is_stop = item
                    for k in range(CH // MM_N):
                        nc.tensor.matmul(
                            out=cls_t[:, k * MM_N : (k + 1) * MM_N],
                            lhsT=ycl[:, _cid * P : (_cid + 1) * P],
                            rhs=e_t[:, k * MM_N : (k + 1) * MM_N],
                            start=is_start,
                            stop=is_stop,
                            skip_group_check=True,
                        )

                jid = 0
                cid = 0
                for G in range(NG):
                    nfull = [0, 2, 4, 6][G]
                    njobs = nfull + 2
                    cls = None
                    pending_cls = []
                    if nfull:
                        cls = clp.tile([P, CH], f32, tag="cls", name=f"cls{G}")
                    for s in range(njobs):
                        e = ep.tile([P, CH], bf16, tag="e", name=f"e{jid}")
                        hw = CH // 2
                        for h in range(2):
                            ps = mpp.tile([P, hw], f32, tag="ps", name=f"ps{jid}_{h}")
                            for k in range(2):
                                j0 = G * CH + h * hw + k * MM_N
                                nc.tensor.matmul(
                                    out=ps[:, k * MM_N : (k + 1) * MM_N],
                                    lhsT=xiT[:, jid * P : (jid + 1) * P],
                                    rhs=xnT[:, j0 : j0 + MM_N],
                                    start=True,
                                    stop=True,
                                )
                            nc.scalar.activation(
                                out=e[:, h * hw : (h + 1) * hw],
                                in_=ps[:],
                                func=AF.Exp,
                                scale=1.0 / TAU,
                                accum_out=outs[:, NJOBS + 2 * jid + h :
                                                NJOBS + 2 * jid + h + 1],
                            )
                        nc.vector.scalar_tensor_tensor(
                            out=trash[:],
                            in0=ybc[:, G * CH : (G + 1) * CH],
                            scalar=yis[:, jid : jid + 1],
                            in1=e[:],
                            op0=OP.is_equal,
                            op1=OP.mult,
                            accum_out=outs[:, jid : jid + 1],
                        )
                        if s < nfull:
                            pending_cls.append((cls, e, cid, s == 0, s == nfull - 1))
                            cid += 1
                        if pending_cls and s >= 1:
                            flush_cls(pending_cls.pop(0))
                        jid += 1
                    while pending_cls:
                        flush_cls(pending_cls.pop(0))
                    if nfull:
                        # copy cls psum -> sbuf bf16 (split across DVE/ACT),
                        # then stream to DRAM; host does the column gather.
                        clsS = csp.tile([P, CH], bf16, tag="clsS", name=f"clsS{G}")
                        nc.vector.tensor_copy(
                            out=clsS[:, : CH // 2], in_=cls[:, : CH // 2]
                        )
                        nc.scalar.copy(
                            out=clsS[:, CH // 2 :], in_=cls[:, CH // 2 :]
                        )
                        nc.sync.dma_start(
                            out=cls_h[:, (G - 1) * CH : G * CH], in_=clsS[:]
                        )
                assert jid == NJOBS and cid == NFULL
            nc.gpsimd.dma_start(out=out_h[:, :], in_=outs[:])
    nc.compile()
    return nc


def _get_program():
    global _PROGRAM
    if _PROGRAM is None:
        _PROGRAM = _build_program()
    return _PROGRAM


def make_in_maps(x, y):
    x = np.asarray(x, dtype=np.float64)
    yf = np.asarray(y).astype(np.float32)
    xn = x / np.linalg.norm(x, axis=-1, keepdims=True)
    xnT = np.ascontiguousarray(xn.T.astype(ml_dtypes.bfloat16))   # [D, N]
    ybc = np.ascontiguousarray(
        np.broadcast_to(yf.astype(ml_dtypes.bfloat16)[None, :], (P, N))
    )
    yi = np.asarray(y).astype(np.int64)
    in_maps = []
    for core in range(NCORES):
        jobs = job_table(core)
        xiT = np.empty((P, NJOBS * P), ml_dtypes.bfloat16)
        yis = np.empty((P, NJOBS), np.float32)
        ycl = np.zeros((P, NFULL * P), ml_dtypes.bfloat16)
        cid = 0
        for jid, (rho, G, kind) in enumerate(jobs):
            rows = slice(rho * P, (rho + 1) * P)
            xiT[:, jid * P : (jid + 1) * P] = xnT[:, rows]
            yis[:, jid] = yf[rows]
            if kind == "full":
                Y = np.zeros((P, P), np.float32)
                Y[np.arange(P), yi[rows]] = 1.0
                Y[:, 100] = 1.0
                ycl[:, cid * P : (cid + 1) * P] = Y.astype(ml_dtypes.bfloat16)
                cid += 1
        in_maps.append(
            {
                "xnT": xnT,
                "xiT": np.ascontiguousarray(xiT),
                "ycls": np.ascontiguousarray(ycl),
                "y_bcast": ybc,
                "y_own": yis,
            }
        )
    return in_maps


def finalize(per_core_outs, per_core_cls, y):
    yi = np.asarray(y).astype(np.int64)
    down = np.zeros(N, np.float64)
    top = np.zeros(N, np.float64)
    for core in range(NCORES):
        o = np.asarray(per_core_outs[core], dtype=np.float64)  # [P, 3*NJOBS]
        cl = np.asarray(per_core_cls[core], dtype=np.float64)  # [P, 3*CH]
        for jid, (rho, G, kind) in enumerate(job_table(core)):
            rows = slice(rho * P, (rho + 1) * P)
            top[rows] += o[:, jid]
            down[rows] += o[:, NJOBS + 2 * jid] + o[:, NJOBS + 2 * jid + 1]
        for G in range(1, 4):
            blk = cl[:, (G - 1) * CH : G * CH]                 # [128 c', 2048 j]
            cols = np.arange(G * CH, (G + 1) * CH)
            down[cols] += blk[100, :]
            top[cols] += blk[yi[cols], np.arange(CH)]
    return np.float32(np.mean(np.log(down) - np.log(top)))


def kernel(x, y):
    from concourse.bass_utils import run_bass_kernel_spmd

    nc = _get_program()
    in_maps = make_in_maps(x, y)
    res = run_bass_kernel_spmd(nc, in_maps, list(range(NCORES)))
    return finalize(
        [r["out"] for r in res.results],
        [r["cls_out"] for r in res.results],
        y,
    )


# revision 15
# speedup vs baseline: 1.6267x; 1.0356x over previous
"""Supervised contrastive loss on 8 Trainium2 NeuronCores — symmetric scheme.

Reference computation (N=8192, D=128, TAU=0.1, 100 classes):
    xn   = x / ||x||_row
    sim  = xn @ xn.T                      [N, N]
    e    = exp(sim / TAU)
    top  = sum_j e[i,j] * (y_i == y_j)
    down = sum_j e[i,j]
    loss = mean(log(down) - log(top))

e is symmetric, so only ~62.5% of the N^2 area is computed. The matrix is
tiled into 64 i-tiles (128 rows) x 4 column groups G (2048 cols). Per
(core, G): [0,2,4,6][G] "full" jobs (i-tiles strictly above G's diagonal
square, dealt round-robin by rho%8) plus 2 diagonal-square jobs -> 20
uniform jobs per core, identical program on every core (per-core data via
inputs only).

Per job: bf16 GEMM -> PSUM [128,2048]; exp on ACT in two 1024-halves
(accum_out = down row-sums); stt on DVE (masked top row-sums). Full jobs
additionally stream e through a class-colsum matmul (one-hot + ones lhsT)
accumulated in a per-G PSUM [128, 2048]; that per-class column-sum block
is copied out and the host gathers top-col = cls[y_j, j], down-col =
cls[100, j] for the mirror (lower-triangle) contributions.

Host does normalization/transpose/bf16-cast of x and the final log/mean.
"""

import sys

import numpy as np

sys.path.insert(0, "/opt/trn_rl_repo")

import ml_dtypes

TAU = 0.1
N, D = 8192, 128
P = 128
NCORES = 8
CH = 2048                   # column group width (4 PSUM banks)
NG = 4                      # column groups
NJOBS = 20                  # jobs per core (uniform)
NFULL = 12                  # full (cls-contributing) jobs per core
MM_N = 512                  # one PSUM bank of f32 per matmul

_PROGRAM = None


def job_table(core):
    """Uniform job list: (rho, G, kind); identical structure for all cores."""
    jobs = []
    for G in (3, 2, 1, 0):
        for r in [r for r in range(16 * G) if r % 8 == core]:
            jobs.append((r, G, "full"))
        for r in (16 * G + 2 * core, 16 * G + 2 * core + 1):
            jobs.append((r, G, "diag"))
    return jobs


# job structure shared by every core: kinds/G per jid
_JOBS0 = job_table(0)
assert len(_JOBS0) == NJOBS
assert sum(1 for _, _, k in _JOBS0 if k == "full") == NFULL


def _build_program():
    import concourse.bacc as bacc
    import concourse.bass as bass  # noqa: F401
    import concourse.mybir as mybir
    from concourse.tile import TileContext

    f32 = mybir.dt.float32
    bf16 = mybir.dt.bfloat16
    AF = mybir.ActivationFunctionType
    OP = mybir.AluOpType

    nc = bacc.Bacc("TRN2", target_bir_lowering=False)
    xnT_h = nc.declare_dram_parameter("xnT", [P, N], bf16, isOutput=False)
    xiT_h = nc.declare_dram_parameter("xiT", [P, NJOBS * P], bf16, isOutput=False)
    ycl_h = nc.declare_dram_parameter("ycls", [P, NJOBS * P], bf16, isOutput=False)
    yb_h = nc.declare_dram_parameter("y_bcast", [P, N], bf16, isOutput=False)
    yi_h = nc.declare_dram_parameter("y_own", [P, NJOBS], f32, isOutput=False)
    out_h = nc.declare_dram_parameter("out", [P, 3 * NJOBS], f32, isOutput=True)
    cls_h = nc.declare_dram_parameter("cls_out", [P, 4 * CH], bf16, isOutput=True)

    with TileContext(nc) as tc:
        with tc.tile_pool(name="persist", bufs=1) as persist:
            xnT = persist.tile([P, N], bf16)
            xiT = persist.tile([P, NJOBS * P], bf16)
            ycl = persist.tile([P, NJOBS * P], bf16)
            ybc = persist.tile([P, N], bf16)
            yis = persist.tile([P, NJOBS], f32)
            outs = persist.tile([P, 3 * NJOBS], f32)

            nc.sync.dma_start(out=xiT[:], in_=xiT_h[:, :])
            first = True
            for g in (3, 2, 1, 0):
                nc.sync.dma_start(
                    out=xnT[:, g * CH : (g + 1) * CH],
                    in_=xnT_h[:, g * CH : (g + 1) * CH],
                )
                nc.gpsimd.dma_start(
                    out=ybc[:, g * CH : (g + 1) * CH],
                    in_=yb_h[:, g * CH : (g + 1) * CH],
                )
                if first:
                    nc.gpsimd.dma_start(out=ycl[:], in_=ycl_h[:, :])
                    nc.sync.dma_start(out=yis[:], in_=yi_h[:, :])
                    first = False

            with (
                tc.tile_pool(name="mpsum", bufs=2, space="PSUM") as mpp,
                tc.tile_pool(name="clsp", bufs=1, space="PSUM") as clp,
                tc.tile_pool(name="ep", bufs=3) as ep,
                tc.tile_pool(name="csb", bufs=2) as csp,
                tc.tile_pool(name="trashp", bufs=1) as trp,
            ):
                trash = trp.tile([P, CH], bf16)

                def flush_cls(item):
                    # cls matmuls delayed one job so the PE never waits on
                    # the exp of the job it just multiplied
                    cls_t, e_t, _cid, is_start, is_stop = item
                    for k in range(CH // MM_N):
                        nc.tensor.matmul(
                            out=cls_t[:, k * MM_N : (k + 1) * MM_N],
                            lhsT=ycl[:, _cid * P : (_cid + 1) * P],
                            rhs=e_t[:, k * MM_N : (k + 1) * MM_N],
                            start=is_start,
                            stop=is_stop,
                            skip_group_check=True,
                        )

                jid = 0
                for G in (3, 2, 1, 0):
                    nfull = [0, 2, 4, 6][G]
                    njobs = nfull + 2
                    pending_cls = []
                    cls = clp.tile([P, CH], f32, tag="cls", name=f"cls{G}")
                    for s in range(njobs):
                        e = ep.tile([P, CH], bf16, tag="e", name=f"e{jid}")
                        hw = CH // 2
                        full = s < nfull
                        for h in range(2):
                            ps = mpp.tile([P, hw], f32, tag="ps", name=f"ps{jid}_{h}")
                            for k in range(2):
                                j0 = G * CH + h * hw + k * MM_N
                                nc.tensor.matmul(
                                    out=ps[:, k * MM_N : (k + 1) * MM_N],
                                    lhsT=xiT[:, jid * P : (jid + 1) * P],
                                    rhs=xnT[:, j0 : j0 + MM_N],
                                    start=True,
                                    stop=True,
                                )
                            nc.scalar.activation(
                                out=e[:, h * hw : (h + 1) * hw],
                                in_=ps[:],
                                func=AF.Exp,
                                scale=1.0 / TAU,
                                accum_out=(outs[:, NJOBS + 2 * jid + h :
                                                NJOBS + 2 * jid + h + 1]
                                           if full else None),
                            )
                        if full:
                            nc.vector.scalar_tensor_tensor(
                                out=trash[:],
                                in0=ybc[:, G * CH : (G + 1) * CH],
                                scalar=yis[:, jid : jid + 1],
                                in1=e[:],
                                op0=OP.is_equal,
                                op1=OP.mult,
                                accum_out=outs[:, jid : jid + 1],
                            )
                        pending_cls.append((cls, e, jid, s == 0, s == njobs - 1))
                        if pending_cls and s >= 1:
                            flush_cls(pending_cls.pop(0))
                        jid += 1
                    while pending_cls:
                        flush_cls(pending_cls.pop(0))
                    # copy cls psum -> sbuf bf16 (split across DVE/ACT),
                    # then stream to DRAM; host does the column gather.
                    clsS = csp.tile([P, CH], bf16, tag="clsS", name=f"clsS{G}")
                    nc.vector.tensor_copy(
                        out=clsS[:, : CH // 2], in_=cls[:, : CH // 2]
                    )
                    nc.scalar.copy(
                        out=clsS[:, CH // 2 :], in_=cls[:, CH // 2 :]
                    )
                    nc.sync.dma_start(
                        out=cls_h[:, G * CH : (G + 1) * CH], in_=clsS[:]
                    )
                assert jid == NJOBS
            nc.gpsimd.dma_start(out=out_h[:, :], in_=outs[:])
    nc.compile()
    return nc


def _get_program():
    global _PROGRAM
    if _PROGRAM is None:
        _PROGRAM = _build_program()
    return _PROGRAM


def make_in_maps(x, y):
    x = np.asarray(x, dtype=np.float64)
    yf = np.asarray(y).astype(np.float32)
    xn = x / np.linalg.norm(x, axis=-1, keepdims=True)
    xnT = np.ascontiguousarray(xn.T.astype(ml_dtypes.bfloat16))   # [D, N]
    ybc = np.ascontiguousarray(
        np.broadcast_to(yf.astype(ml_dtypes.bfloat16)[None, :], (P, N))
    )
    yi = np.asarray(y).astype(np.int64)
    in_maps = []
    for core in range(NCORES):
        jobs = job_table(core)
        xiT = np.empty((P, NJOBS * P), ml_dtypes.bfloat16)
        yis = np.empty((P, NJOBS), np.float32)
        ycl = np.zeros((P, NJOBS * P), ml_dtypes.bfloat16)
        for jid, (rho, G, kind) in enumerate(jobs):
            rows = slice(rho * P, (rho + 1) * P)
            xiT[:, jid * P : (jid + 1) * P] = xnT[:, rows]
            yis[:, jid] = yf[rows]
            Y = np.zeros((P, P), np.float32)
            Y[np.arange(P), yi[rows]] = 1.0
            Y[:, 100] = 1.0
            ycl[:, jid * P : (jid + 1) * P] = Y.astype(ml_dtypes.bfloat16)
        in_maps.append(
            {
                "xnT": xnT,
                "xiT": np.ascontiguousarray(xiT),
                "ycls": np.ascontiguousarray(ycl),
                "y_bcast": ybc,
                "y_own": yis,
            }
        )
    return in_maps


def finalize(per_core_outs, per_core_cls, y):
    yi = np.asarray(y).astype(np.int64)
    down = np.zeros(N, np.float64)
    top = np.zeros(N, np.float64)
    for core in range(NCORES):
        o = np.asarray(per_core_outs[core], dtype=np.float64)  # [P, 3*NJOBS]
        cl = np.asarray(per_core_cls[core], dtype=np.float64)  # [P, 4*CH]
        for jid, (rho, G, kind) in enumerate(job_table(core)):
            if kind != "full":
                continue
            rows = slice(rho * P, (rho + 1) * P)
            top[rows] += o[:, jid]
            down[rows] += o[:, NJOBS + 2 * jid] + o[:, NJOBS + 2 * jid + 1]
        for G in range(4):
            blk = cl[:, G * CH : (G + 1) * CH]                 # [128 c', 2048 j]
            cols = np.arange(G * CH, (G + 1) * CH)
            down[cols] += blk[100, :]
            top[cols] += blk[yi[cols], np.arange(CH)]
    return np.float32(np.mean(np.log(down) - np.log(top)))


def kernel(x, y):
    from concourse.bass_utils import run_bass_kernel_spmd

    nc = _get_program()
    in_maps = make_in_maps(x, y)
    res = run_bass_kernel_spmd(nc, in_maps, list(range(NCORES)))
    return finalize(
        [r["out"] for r in res.results],
        [r["cls_out"] for r in res.results],
        y,
    )


# revision 16
# speedup vs baseline: 1.6477x; 1.0129x over previous
"""Supervised contrastive loss on 8 Trainium2 NeuronCores — symmetric scheme.

Reference computation (N=8192, D=128, TAU=0.1, 100 classes):
    xn   = x / ||x||_row
    sim  = xn @ xn.T                      [N, N]
    e    = exp(sim / TAU)
    top  = sum_j e[i,j] * (y_i == y_j)
    down = sum_j e[i,j]
    loss = mean(log(down) - log(top))

e is symmetric, so only ~62.5% of the N^2 area is computed. The matrix is
tiled into 64 i-tiles (128 rows) x 4 column groups G (2048 cols). Per
(core, G): [0,2,4,6][G] "full" jobs (i-tiles strictly above G's diagonal
square, dealt round-robin by rho%8) plus 2 diagonal-square jobs -> 20
uniform jobs per core, identical program on every core (per-core data via
inputs only).

Per job: bf16 GEMM -> PSUM [128,2048]; exp on ACT in two 1024-halves
(accum_out = down row-sums); stt on DVE (masked top row-sums). Full jobs
additionally stream e through a class-colsum matmul (one-hot + ones lhsT)
accumulated in a per-G PSUM [128, 2048]; that per-class column-sum block
is copied out and the host gathers top-col = cls[y_j, j], down-col =
cls[100, j] for the mirror (lower-triangle) contributions.

Host does normalization/transpose/bf16-cast of x and the final log/mean.
"""

import sys

import numpy as np

sys.path.insert(0, "/opt/trn_rl_repo")

import ml_dtypes

TAU = 0.1
N, D = 8192, 128
P = 128
NCORES = 8
CH = 2048                   # column group width (4 PSUM banks)
NG = 4                      # column groups
NJOBS = 20                  # jobs per core (uniform)
NFULL = 12                  # full (cls-contributing) jobs per core
MM_N = 512                  # one PSUM bank of f32 per matmul

_PROGRAM = None


def job_table(core):
    """Uniform job list: (rho, G, kind); identical structure for all cores."""
    jobs = []
    for G in (3, 2, 1, 0):
        for r in [r for r in range(16 * G) if r % 8 == core]:
            jobs.append((r, G, "full"))
        for r in (16 * G + 2 * core, 16 * G + 2 * core + 1):
            jobs.append((r, G, "diag"))
    return jobs


# job structure shared by every core: kinds/G per jid
_JOBS0 = job_table(0)
assert len(_JOBS0) == NJOBS
assert sum(1 for _, _, k in _JOBS0 if k == "full") == NFULL


def _build_program():
    import concourse.bacc as bacc
    import concourse.bass as bass  # noqa: F401
    import concourse.mybir as mybir
    from concourse.tile import TileContext

    f32 = mybir.dt.float32
    bf16 = mybir.dt.bfloat16
    AF = mybir.ActivationFunctionType
    OP = mybir.AluOpType

    nc = bacc.Bacc("TRN2", target_bir_lowering=False)
    xnT_h = nc.declare_dram_parameter("xnT", [P, N], bf16, isOutput=False)
    xiT_h = nc.declare_dram_parameter("xiT", [P, NJOBS * P], bf16, isOutput=False)
    ycl_h = nc.declare_dram_parameter("ycls", [P, NJOBS * P], bf16, isOutput=False)
    yb_h = nc.declare_dram_parameter("y_bcast", [P, N], bf16, isOutput=False)
    yi_h = nc.declare_dram_parameter("y_own", [P, NJOBS], f32, isOutput=False)
    out_h = nc.declare_dram_parameter("out", [P, 3 * NJOBS], f32, isOutput=True)
    cls_h = nc.declare_dram_parameter("cls_out", [P, 4 * CH], bf16, isOutput=True)

    with TileContext(nc) as tc:
        with tc.tile_pool(name="persist", bufs=1) as persist:
            xnT = [persist.tile([P, CH], bf16, name=f"xnT{g}") for g in range(NG)]
            xiT = persist.tile([P, NJOBS * P], bf16)
            ycl = persist.tile([P, NJOBS * P], bf16)
            ybc = [persist.tile([P, CH], bf16, name=f"ybc{g}") for g in range(NG)]
            yis = persist.tile([P, NJOBS], f32)
            outs = persist.tile([P, 3 * NJOBS], f32)

            nc.sync.dma_start(out=xiT[:], in_=xiT_h[:, :])
            first = True
            for g in (3, 2, 1, 0):
                nc.sync.dma_start(
                    out=xnT[g][:], in_=xnT_h[:, g * CH : (g + 1) * CH]
                )
                nc.gpsimd.dma_start(
                    out=ybc[g][:], in_=yb_h[:, g * CH : (g + 1) * CH]
                )
                if first:
                    nc.gpsimd.dma_start(out=ycl[:], in_=ycl_h[:, :])
                    nc.sync.dma_start(out=yis[:], in_=yi_h[:, :])
                    first = False

            with (
                tc.tile_pool(name="mpsum", bufs=2, space="PSUM") as mpp,
                tc.tile_pool(name="clsp", bufs=1, space="PSUM") as clp,
                tc.tile_pool(name="ep", bufs=3) as ep,
                tc.tile_pool(name="csb", bufs=2) as csp,
                tc.tile_pool(name="trashp", bufs=1) as trp,
            ):
                trash = trp.tile([P, CH], bf16)

                def flush_cls(item):
                    # cls matmuls delayed one job so the PE never waits on
                    # the exp of the job it just multiplied
                    cls_t, e_t, _cid, is_start, is_stop = item
                    for k in range(CH // MM_N):
                        nc.tensor.matmul(
                            out=cls_t[:, k * MM_N : (k + 1) * MM_N],
                            lhsT=ycl[:, _cid * P : (_cid + 1) * P],
                            rhs=e_t[:, k * MM_N : (k + 1) * MM_N],
                            start=is_start,
                            stop=is_stop,
                            skip_group_check=True,
                        )

                jid = 0
                for G in (3, 2, 1, 0):
                    nfull = [0, 2, 4, 6][G]
                    njobs = nfull + 2
                    pending_cls = []
                    cls = clp.tile([P, CH], f32, tag="cls", name=f"cls{G}")
                    for s in range(njobs):
                        e = ep.tile([P, CH], bf16, tag="e", name=f"e{jid}")
                        hw = CH // 2
                        full = s < nfull
                        for h in range(2):
                            ps = mpp.tile([P, hw], f32, tag="ps", name=f"ps{jid}_{h}")
                            for k in range(2):
                                j0 = G * CH + h * hw + k * MM_N
                                nc.tensor.matmul(
                                    out=ps[:, k * MM_N : (k + 1) * MM_N],
                                    lhsT=xiT[:, jid * P : (jid + 1) * P],
                                    rhs=xnT[G][:, j0 - G * CH : j0 - G * CH + MM_N],
                                    start=True,
                                    stop=True,
                                )
                            nc.scalar.activation(
                                out=e[:, h * hw : (h + 1) * hw],
                                in_=ps[:],
                                func=AF.Exp,
                                scale=1.0 / TAU,
                                accum_out=(outs[:, NJOBS + 2 * jid + h :
                                                NJOBS + 2 * jid + h + 1]
                                           if full else None),
                            )
                        if full:
                            nc.vector.scalar_tensor_tensor(
                                out=trash[:],
                                in0=ybc[G][:],
                                scalar=yis[:, jid : jid + 1],
                                in1=e[:],
                                op0=OP.is_equal,
                                op1=OP.mult,
                                accum_out=outs[:, jid : jid + 1],
                            )
                        pending_cls.append((cls, e, jid, s == 0, s == njobs - 1))
                        if pending_cls and s >= 1:
                            flush_cls(pending_cls.pop(0))
                        jid += 1
                    while pending_cls:
                        flush_cls(pending_cls.pop(0))
                    # copy cls psum -> sbuf bf16 (split across DVE/ACT),
                    # then stream to DRAM; host does the column gather.
                    clsS = csp.tile([P, CH], bf16, tag="clsS", name=f"clsS{G}")
                    nc.vector.tensor_copy(
                        out=clsS[:, : CH // 2], in_=cls[:, : CH // 2]
                    )
                    nc.vector.tensor_copy(
                        out=clsS[:, CH // 2 :], in_=cls[:, CH // 2 :]
                    )
                    nc.sync.dma_start(
                        out=cls_h[:, G * CH : (G + 1) * CH], in_=clsS[:]
                    )
                assert jid == NJOBS
            nc.gpsimd.dma_start(out=out_h[:, :], in_=outs[:])
    nc.compile()
    return nc


def _get_program():
    global _PROGRAM
    if _PROGRAM is None:
        _PROGRAM = _build_program()
    return _PROGRAM


def make_in_maps(x, y):
    x = np.asarray(x, dtype=np.float64)
    yf = np.asarray(y).astype(np.float32)
    xn = x / np.linalg.norm(x, axis=-1, keepdims=True)
    xnT = np.ascontiguousarray(xn.T.astype(ml_dtypes.bfloat16))   # [D, N]
    ybc = np.ascontiguousarray(
        np.broadcast_to(yf.astype(ml_dtypes.bfloat16)[None, :], (P, N))
    )
    yi = np.asarray(y).astype(np.int64)
    in_maps = []
    for core in range(NCORES):
        jobs = job_table(core)
        xiT = np.empty((P, NJOBS * P), ml_dtypes.bfloat16)
        yis = np.empty((P, NJOBS), np.float32)
        ycl = np.zeros((P, NJOBS * P), ml_dtypes.bfloat16)
        for jid, (rho, G, kind) in enumerate(jobs):
            rows = slice(rho * P, (rho + 1) * P)
            xiT[:, jid * P : (jid + 1) * P] = xnT[:, rows]
            yis[:, jid] = yf[rows]
            Y = np.zeros((P, P), np.float32)
            Y[np.arange(P), yi[rows]] = 1.0
            Y[:, 100] = 1.0
            ycl[:, jid * P : (jid + 1) * P] = Y.astype(ml_dtypes.bfloat16)
        in_maps.append(
            {
                "xnT": xnT,
                "xiT": np.ascontiguousarray(xiT),
                "ycls": np.ascontiguousarray(ycl),
                "y_bcast": ybc,
                "y_own": yis,
            }
        )
    return in_maps


def finalize(per_core_outs, per_core_cls, y):
    yi = np.asarray(y).astype(np.int64)
    down = np.zeros(N, np.float64)
    top = np.zeros(N, np.float64)
    for core in range(NCORES):
        o = np.asarray(per_core_outs[core], dtype=np.float64)  # [P, 3*NJOBS]
        cl = np.asarray(per_core_cls[core], dtype=np.float64)  # [P, 4*CH]
        for jid, (rho, G, kind) in enumerate(job_table(core)):
            if kind != "full":
                continue
            rows = slice(rho * P, (rho + 1) * P)
            top[rows] += o[:, jid]
            down[rows] += o[:, NJOBS + 2 * jid] + o[:, NJOBS + 2 * jid + 1]
        for G in range(4):
            blk = cl[:, G * CH : (G + 1) * CH]                 # [128 c', 2048 j]
            cols = np.arange(G * CH, (G + 1) * CH)
            down[cols] += blk[100, :]
            top[cols] += blk[yi[cols], np.arange(CH)]
    return np.float32(np.mean(np.log(down) - np.log(top)))


def kernel(x, y):
    from concourse.bass_utils import run_bass_kernel_spmd

    nc = _get_program()
    in_maps = make_in_maps(x, y)
    res = run_bass_kernel_spmd(nc, in_maps, list(range(NCORES)))
    return finalize(
        [r["out"] for r in res.results],
        [r["cls_out"] for r in res.results],
        y,
    )


# revision 17
# speedup vs baseline: 1.7724x; 1.0757x over previous
"""Supervised contrastive loss on 8 Trainium2 NeuronCores — symmetric scheme.

Reference computation (N=8192, D=128, TAU=0.1, 100 classes):
    xn   = x / ||x||_row
    sim  = xn @ xn.T                      [N, N]
    e    = exp(sim / TAU)
    top  = sum_j e[i,j] * (y_i == y_j)
    down = sum_j e[i,j]
    loss = mean(log(down) - log(top))

e is symmetric, so only ~62.5% of the N^2 area is computed. The matrix is
tiled into 64 i-tiles (128 rows) x 4 column groups G (2048 cols). Per
(core, G): [0,2,4,6][G] "full" jobs (i-tiles strictly above G's diagonal
square, dealt round-robin by rho%8) plus 2 diagonal-square jobs -> 20
uniform jobs per core, identical program on every core (per-core data via
inputs only).

Per job: bf16 GEMM -> PSUM [128,2048]; exp on ACT in two 1024-halves
(accum_out = down row-sums); stt on DVE (masked top row-sums). Full jobs
additionally stream e through a class-colsum matmul (one-hot + ones lhsT)
accumulated in a per-G PSUM [128, 2048]; that per-class column-sum block
is copied out and the host gathers top-col = cls[y_j, j], down-col =
cls[100, j] for the mirror (lower-triangle) contributions.

Host does normalization/transpose/bf16-cast of x and the final log/mean.
"""

import sys

import numpy as np

sys.path.insert(0, "/opt/trn_rl_repo")

import ml_dtypes

TAU = 0.1
N, D = 8192, 128
P = 128
NCORES = 8
CH = 2048                   # column group width (4 PSUM banks)
NG = 4                      # column groups
NJOBS = 20                  # jobs per core (uniform)
NFULL = 12                  # full (cls-contributing) jobs per core
MM_N = 512                  # one PSUM bank of f32 per matmul

_PROGRAM = None


def job_table(core):
    """Uniform job list: (rho, G, kind); identical structure for all cores."""
    jobs = []
    for G in (3, 2, 1, 0):
        for r in [r for r in range(16 * G) if r % 8 == core]:
            jobs.append((r, G, "full"))
        for r in (16 * G + 2 * core, 16 * G + 2 * core + 1):
            jobs.append((r, G, "diag"))
    return jobs


# job structure shared by every core: kinds/G per jid
_JOBS0 = job_table(0)
assert len(_JOBS0) == NJOBS
assert sum(1 for _, _, k in _JOBS0 if k == "full") == NFULL


def _build_program():
    import concourse.bacc as bacc
    import concourse.bass as bass  # noqa: F401
    import concourse.mybir as mybir
    from concourse.tile import TileContext

    f32 = mybir.dt.float32
    bf16 = mybir.dt.bfloat16
    AF = mybir.ActivationFunctionType
    OP = mybir.AluOpType

    nc = bacc.Bacc("TRN2", target_bir_lowering=False)
    xnT_h = nc.declare_dram_parameter("xnT", [P, N], bf16, isOutput=False)
    xiT_h = nc.declare_dram_parameter("xiT", [P, NJOBS * P], bf16, isOutput=False)
    ycl_h = nc.declare_dram_parameter("ycls", [P, NJOBS * P], bf16, isOutput=False)
    i8 = mybir.dt.int8
    yb_h = nc.declare_dram_parameter("y_bcast", [P, N], i8, isOutput=False)
    yi_h = nc.declare_dram_parameter("y_own", [P, NJOBS], f32, isOutput=False)
    out_h = nc.declare_dram_parameter("out", [P, 3 * NJOBS], f32, isOutput=True)
    cls_h = nc.declare_dram_parameter("cls_out", [P, 4 * CH], bf16, isOutput=True)

    with TileContext(nc) as tc:
        with tc.tile_pool(name="persist", bufs=1) as persist:
            xnT = [persist.tile([P, CH], bf16, name=f"xnT{g}") for g in range(NG)]
            xiT = persist.tile([P, NJOBS * P], bf16)
            ycl = persist.tile([P, NJOBS * P], bf16)
            ybc = [persist.tile([P, CH], i8, name=f"ybc{g}") for g in range(NG)]
            yis = persist.tile([P, NJOBS], f32)
            outs = persist.tile([P, 3 * NJOBS], f32)

            HD = 4 * P  # first 4 jobs' stationary data
            nc.sync.dma_start(out=xiT[:, :HD], in_=xiT_h[:, :HD])
            nc.sync.dma_start(out=yis[:], in_=yi_h[:, :])
            nc.gpsimd.dma_start(out=ycl[:, :HD], in_=ycl_h[:, :HD])
            first = True
            for g in (3, 2, 1, 0):
                nc.sync.dma_start(
                    out=xnT[g][:], in_=xnT_h[:, g * CH : (g + 1) * CH]
                )
                nc.gpsimd.dma_start(
                    out=ybc[g][:], in_=yb_h[:, g * CH : (g + 1) * CH]
                )
                if first:
                    nc.sync.dma_start(out=xiT[:, HD:], in_=xiT_h[:, HD:])
                    nc.gpsimd.dma_start(out=ycl[:, HD:], in_=ycl_h[:, HD:])
                    first = False

            with (
                tc.tile_pool(name="mpsum", bufs=2, space="PSUM") as mpp,
                tc.tile_pool(name="clsp", bufs=1, space="PSUM") as clp,
                tc.tile_pool(name="ep", bufs=3) as ep,
                tc.tile_pool(name="csb", bufs=2) as csp,
                tc.tile_pool(name="trashp", bufs=1) as trp,
            ):
                trash = trp.tile([P, CH], bf16)

                def flush_cls(item):
                    # cls matmuls delayed one job so the PE never waits on
                    # the exp of the job it just multiplied
                    cls_t, e_t, _cid, is_start, is_stop = item
                    for k in range(CH // MM_N):
                        nc.tensor.matmul(
                            out=cls_t[:, k * MM_N : (k + 1) * MM_N],
                            lhsT=ycl[:, _cid * P : (_cid + 1) * P],
                            rhs=e_t[:, k * MM_N : (k + 1) * MM_N],
                            start=is_start,
                            stop=is_stop,
                            skip_group_check=True,
                        )

                jid = 0
                for G in (3, 2, 1, 0):
                    nfull = [0, 2, 4, 6][G]
                    njobs = nfull + 2
                    pending_cls = []
                    cls = clp.tile([P, CH], f32, tag="cls", name=f"cls{G}")
                    for s in range(njobs):
                        e = ep.tile([P, CH], bf16, tag="e", name=f"e{jid}")
                        hw = CH // 2
                        full = s < nfull
                        for h in range(2):
                            ps = mpp.tile([P, hw], f32, tag="ps", name=f"ps{jid}_{h}")
                            for k in range(2):
                                j0 = G * CH + h * hw + k * MM_N
                                nc.tensor.matmul(
                                    out=ps[:, k * MM_N : (k + 1) * MM_N],
                                    lhsT=xiT[:, jid * P : (jid + 1) * P],
                                    rhs=xnT[G][:, j0 - G * CH : j0 - G * CH + MM_N],
                                    start=True,
                                    stop=True,
                                )
                            nc.scalar.activation(
                                out=e[:, h * hw : (h + 1) * hw],
                                in_=ps[:],
                                func=AF.Exp,
                                scale=1.0 / TAU,
                                accum_out=(outs[:, NJOBS + 2 * jid + h :
                                                NJOBS + 2 * jid + h + 1]
                                           if full else None),
                            )
                        if full:
                            nc.vector.scalar_tensor_tensor(
                                out=trash[:],
                                in0=ybc[G][:],
                                scalar=yis[:, jid : jid + 1],
                                in1=e[:],
                                op0=OP.is_equal,
                                op1=OP.mult,
                                accum_out=outs[:, jid : jid + 1],
                            )
                        pending_cls.append((cls, e, jid, s == 0, s == njobs - 1))
                        if pending_cls and s >= 1:
                            flush_cls(pending_cls.pop(0))
                        jid += 1
                    while pending_cls:
                        flush_cls(pending_cls.pop(0))
                    # copy cls psum -> sbuf bf16 (split across DVE/ACT),
                    # then stream to DRAM; host does the column gather.
                    clsS = csp.tile([P, CH], bf16, tag="clsS", name=f"clsS{G}")
                    for hh in range(2):
                        sl = slice(hh * CH // 2, (hh + 1) * CH // 2)
                        nc.vector.tensor_copy(out=clsS[:, sl], in_=cls[:, sl])
                        nc.sync.dma_start(
                            out=cls_h[:, G * CH + hh * CH // 2 :
                                      G * CH + (hh + 1) * CH // 2],
                            in_=clsS[:, sl],
                        )
                assert jid == NJOBS
            nc.gpsimd.dma_start(out=out_h[:, :], in_=outs[:])
    nc.compile()
    return nc


def _get_program():
    global _PROGRAM
    if _PROGRAM is None:
        _PROGRAM = _build_program()
    return _PROGRAM


def make_in_maps(x, y):
    x = np.asarray(x, dtype=np.float64)
    yf = np.asarray(y).astype(np.float32)
    xn = x / np.linalg.norm(x, axis=-1, keepdims=True)
    xnT = np.ascontiguousarray(xn.T.astype(ml_dtypes.bfloat16))   # [D, N]
    ybc = np.ascontiguousarray(
        np.broadcast_to(np.asarray(y).astype(np.int8)[None, :], (P, N))
    )
    yi = np.asarray(y).astype(np.int64)
    in_maps = []
    for core in range(NCORES):
        jobs = job_table(core)
        xiT = np.empty((P, NJOBS * P), ml_dtypes.bfloat16)
        yis = np.empty((P, NJOBS), np.float32)
        ycl = np.zeros((P, NJOBS * P), ml_dtypes.bfloat16)
        for jid, (rho, G, kind) in enumerate(jobs):
            rows = slice(rho * P, (rho + 1) * P)
            xiT[:, jid * P : (jid + 1) * P] = xnT[:, rows]
            yis[:, jid] = yf[rows]
            Y = np.zeros((P, P), np.float32)
            Y[np.arange(P), yi[rows]] = 1.0
            Y[:, 100] = 1.0
            ycl[:, jid * P : (jid + 1) * P] = Y.astype(ml_dtypes.bfloat16)
        in_maps.append(
            {
                "xnT": xnT,
                "xiT": np.ascontiguousarray(xiT),
                "ycls": np.ascontiguousarray(ycl),
                "y_bcast": ybc,
                "y_own": yis,
            }
        )
    return in_maps


def finalize(per_core_outs, per_core_cls, y):
    yi = np.asarray(y).astype(np.int64)
    down = np.zeros(N, np.float64)
    top = np.zeros(N, np.float64)
    for core in range(NCORES):
        o = np.asarray(per_core_outs[core], dtype=np.float64)  # [P, 3*NJOBS]
        cl = np.asarray(per_core_cls[core], dtype=np.float64)  # [P, 4*CH]
        for jid, (rho, G, kind) in enumerate(job_table(core)):
            if kind != "full":
                continue
            rows = slice(rho * P, (rho + 1) * P)
            top[rows] += o[:, jid]
            down[rows] += o[:, NJOBS + 2 * jid] + o[:, NJOBS + 2 * jid + 1]
        for G in range(4):
            blk = cl[:, G * CH : (G + 1) * CH]                 # [128 c', 2048 j]
            cols = np.arange(G * CH, (G + 1) * CH)
            down[cols] += blk[100, :]
            top[cols] += blk[yi[cols], np.arange(CH)]
    return np.float32(np.mean(np.log(down) - np.log(top)))


def kernel(x, y):
    from concourse.bass_utils import run_bass_kernel_spmd

    nc = _get_program()
    in_maps = make_in_maps(x, y)
    res = run_bass_kernel_spmd(nc, in_maps, list(range(NCORES)))
    return finalize(
        [r["out"] for r in res.results],
        [r["cls_out"] for r in res.results],
        y,
    )


# revision 18
# speedup vs baseline: 1.7834x; 1.0062x over previous
"""Supervised contrastive loss on 8 Trainium2 NeuronCores — symmetric scheme.

Reference computation (N=8192, D=128, TAU=0.1, 100 classes):
    xn   = x / ||x||_row
    sim  = xn @ xn.T                      [N, N]
    e    = exp(sim / TAU)
    top  = sum_j e[i,j] * (y_i == y_j)
    down = sum_j e[i,j]
    loss = mean(log(down) - log(top))

e is symmetric, so only ~56% of the N^2 area is computed. The matrix is
tiled into 64 i-tiles (128 rows) x 4 column groups G (2048 cols). Per
(core, G), uniformly across cores:
  - [0,2,4,6][G] "full" jobs: i-tiles strictly above G's diagonal square
    (dealt round-robin by rho%8), full 2048 width. Row-path (exp accum for
    down, DVE stt for top) plus class-colsum matmul contribution.
  - the 2048x2048 diagonal square is split into quadrants: A (top-left
    1024^2, symmetric), D (bottom-right 1024^2, symmetric), B (top-right
    off-diagonal). Per core: one B job (row-path + cls into the D column
    half), one A and one D job (cls only — the colsums of a symmetric
    block ARE its row sums, so no row-path needed).

The per-G class-colsum PSUM [128(classes+ones), 2048] is copied out and
the host gathers top-col = cls[y_j, j] and down-col = cls[100, j], which
supply all mirror (lower-triangle) contributions. Host does the
normalization/transpose/bf16-cast of x and the final log/mean.
"""

import sys

import numpy as np

sys.path.insert(0, "/opt/trn_rl_repo")

import ml_dtypes

TAU = 0.1
N, D = 8192, 128
P = 128
NCORES = 8
CH = 2048                   # column group width (4 PSUM banks)
HW2 = CH // 2               # quadrant width
NG = 4                      # column groups
NJOBS = 24                  # jobs per core (uniform): 12 full + 4*(B,A,D)
MM_N = 512                  # one PSUM bank of f32 per matmul

_PROGRAM = None


def job_table(core):
    """Uniform job list: (rho, G, kind, half); same structure on every core.

    kind: 'full' (2048 wide), 'B'/'A'/'D' (1024 wide, half selects cols).
    """
    jobs = []
    for G in (3, 2, 1, 0):
        for r in [r for r in range(16 * G) if r % 8 == core]:
            jobs.append((r, G, "full", None))
        jobs.append((16 * G + core, G, "B", 1))
        jobs.append((16 * G + core, G, "A", 0))
        jobs.append((16 * G + 8 + core, G, "D", 1))
    return jobs


_JOBS0 = job_table(0)
assert len(_JOBS0) == NJOBS


def _build_program():
    import concourse.bacc as bacc
    import concourse.bass as bass  # noqa: F401
    import concourse.mybir as mybir
    from concourse.tile import TileContext

    f32 = mybir.dt.float32
    bf16 = mybir.dt.bfloat16
    i8 = mybir.dt.int8
    AF = mybir.ActivationFunctionType
    OP = mybir.AluOpType

    nc = bacc.Bacc("TRN2", target_bir_lowering=False)
    xnT_h = nc.declare_dram_parameter("xnT", [P, N], bf16, isOutput=False)
    xiT_h = nc.declare_dram_parameter("xiT", [P, NJOBS * P], bf16, isOutput=False)
    ycl_h = nc.declare_dram_parameter("ycls", [P, NJOBS * P], bf16, isOutput=False)
    yb_h = nc.declare_dram_parameter("y_bcast", [P, N], i8, isOutput=False)
    yi_h = nc.declare_dram_parameter("y_own", [P, NJOBS], f32, isOutput=False)
    out_h = nc.declare_dram_parameter("out", [P, 3 * NJOBS], f32, isOutput=True)
    cls_h = nc.declare_dram_parameter("cls_out", [P, 4 * CH], bf16, isOutput=True)

    with TileContext(nc) as tc:
        with tc.tile_pool(name="persist", bufs=1) as persist:
            xnT = [persist.tile([P, CH], bf16, name=f"xnT{g}") for g in range(NG)]
            xiT = persist.tile([P, NJOBS * P], bf16)
            ycl = persist.tile([P, NJOBS * P], bf16)
            ybc = [persist.tile([P, CH], i8, name=f"ybc{g}") for g in range(NG)]
            yis = persist.tile([P, NJOBS], f32)
            outs = persist.tile([P, 3 * NJOBS], f32)

            HD = 4 * P  # first jobs' stationary data, loaded first
            nc.sync.dma_start(out=xiT[:, :HD], in_=xiT_h[:, :HD])
            nc.sync.dma_start(out=yis[:], in_=yi_h[:, :])
            nc.gpsimd.dma_start(out=ycl[:, :HD], in_=ycl_h[:, :HD])
            first = True
            for g in (3, 2, 1, 0):
                nc.sync.dma_start(
                    out=xnT[g][:], in_=xnT_h[:, g * CH : (g + 1) * CH]
                )
                nc.gpsimd.dma_start(
                    out=ybc[g][:], in_=yb_h[:, g * CH : (g + 1) * CH]
                )
                if first:
                    nc.sync.dma_start(out=xiT[:, HD:], in_=xiT_h[:, HD:])
                    nc.gpsimd.dma_start(out=ycl[:, HD:], in_=ycl_h[:, HD:])
                    first = False

            with (
                tc.tile_pool(name="mpsum", bufs=2, space="PSUM") as mpp,
                tc.tile_pool(name="clsp", bufs=1, space="PSUM") as clp,
                tc.tile_pool(name="ep", bufs=4) as ep,
                tc.tile_pool(name="csb", bufs=2) as csp,
                tc.tile_pool(name="trashp", bufs=1) as trp,
            ):
                trash = trp.tile([P, CH], bf16)

                def flush_cls(item):
                    # cls matmuls delayed one job so the PE never waits on
                    # the exp of the job it just multiplied
                    cls_t, e_t, jid_, banks, flags = item
                    for bi, k in enumerate(banks):
                        st, sp = flags[k]
                        nc.tensor.matmul(
                            out=cls_t[:, k * MM_N : (k + 1) * MM_N],
                            lhsT=ycl[:, jid_ * P : (jid_ + 1) * P],
                            rhs=e_t[:, bi * MM_N : (bi + 1) * MM_N],
                            start=st,
                            stop=sp,
                            skip_group_check=True,
                        )

                jid = 0
                for G in (3, 2, 1, 0):
                    nfull = [0, 2, 4, 6][G]
                    njobs = nfull + 3
                    kinds = ["full"] * nfull + ["B", "A", "D"]
                    # per-bank contributor order for start/stop flags
                    bank_seq = {k: [] for k in range(4)}
                    for s, kind in enumerate(kinds):
                        bk = ([0, 1, 2, 3] if kind == "full"
                              else [0, 1] if kind == "A" else [2, 3])
                        for k in bk:
                            bank_seq[k].append(s)
                    pending_cls = []
                    cls = clp.tile([P, CH], f32, tag="cls", name=f"cls{G}")
                    for s in range(njobs):
                        kind = kinds[s]
                        if kind == "full":
                            banks = [0, 1, 2, 3]
                            width = CH
                            coff = 0
                        else:
                            banks = [0, 1] if kind == "A" else [2, 3]
                            width = HW2
                            coff = 0 if kind == "A" else HW2
                        e = ep.tile([P, width], bf16,
                                    tag="e" if width == CH else "e1",
                                    name=f"e{jid}")
                        nhalf = width // HW2
                        for h in range(nhalf):
                            ps = mpp.tile([P, HW2], f32, tag="ps",
                                          name=f"ps{jid}_{h}")
                            for k in range(2):
                                j0 = coff + h * HW2 + k * MM_N
                                nc.tensor.matmul(
                                    out=ps[:, k * MM_N : (k + 1) * MM_N],
                                    lhsT=xiT[:, jid * P : (jid + 1) * P],
                                    rhs=xnT[G][:, j0 : j0 + MM_N],
                                    start=True,
                                    stop=True,
                                )
                            nc.scalar.activation(
                                out=e[:, h * HW2 : (h + 1) * HW2],
                                in_=ps[:],
                                func=AF.Exp,
                                scale=1.0 / TAU,
                                accum_out=(outs[:, NJOBS + 2 * jid + h :
                                                NJOBS + 2 * jid + h + 1]
                                           if kind in ("full", "B") else None),
                            )
                        if kind in ("full", "B"):
                            nc.vector.scalar_tensor_tensor(
                                out=trash[:, :width],
                                in0=ybc[G][:, coff : coff + width],
                                scalar=yis[:, jid : jid + 1],
                                in1=e[:],
                                op0=OP.is_equal,
                                op1=OP.mult,
                                accum_out=outs[:, jid : jid + 1],
                            )
                        flags = {
                            k: (bank_seq[k][0] == s, bank_seq[k][-1] == s)
                            for k in banks
                        }
                        pending_cls.append((cls, e, jid, banks, flags))
                        if pending_cls and s >= 1:
                            flush_cls(pending_cls.pop(0))
                        jid += 1
                    while pending_cls:
                        flush_cls(pending_cls.pop(0))
                    # copy cls psum -> sbuf bf16 on DVE, stream to DRAM;
                    # host does the column gather.
                    clsS = csp.tile([P, CH], bf16, tag="clsS", name=f"clsS{G}")
                    for hh in range(2):
                        sl = slice(hh * HW2, (hh + 1) * HW2)
                        nc.vector.tensor_copy(out=clsS[:, sl], in_=cls[:, sl])
                        nc.sync.dma_start(
                            out=cls_h[:, G * CH + hh * HW2 :
                                      G * CH + (hh + 1) * HW2],
                            in_=clsS[:, sl],
                        )
                assert jid == NJOBS
            nc.gpsimd.dma_start(out=out_h[:, :], in_=outs[:])
    nc.compile()
    return nc


def _get_program():
    global _PROGRAM
    if _PROGRAM is None:
        _PROGRAM = _build_program()
    return _PROGRAM


def make_in_maps(x, y):
    x = np.asarray(x, dtype=np.float64)
    yf = np.asarray(y).astype(np.float32)
    xn = x / np.linalg.norm(x, axis=-1, keepdims=True)
    xnT = np.ascontiguousarray(xn.T.astype(ml_dtypes.bfloat16))   # [D, N]
    ybc = np.ascontiguousarray(
        np.broadcast_to(np.asarray(y).astype(np.int8)[None, :], (P, N))
    )
    yi = np.asarray(y).astype(np.int64)
    in_maps = []
    for core in range(NCORES):
        jobs = job_table(core)
        xiT = np.empty((P, NJOBS * P), ml_dtypes.bfloat16)
        yis = np.empty((P, NJOBS), np.float32)
        ycl = np.zeros((P, NJOBS * P), ml_dtypes.bfloat16)
        for jid, (rho, G, kind, half) in enumerate(jobs):
            rows = slice(rho * P, (rho + 1) * P)
            xiT[:, jid * P : (jid + 1) * P] = xnT[:, rows]
            yis[:, jid] = yf[rows]
            Y = np.zeros((P, P), np.float32)
            Y[np.arange(P), yi[rows]] = 1.0
            Y[:, 100] = 1.0
            ycl[:, jid * P : (jid + 1) * P] = Y.astype(ml_dtypes.bfloat16)
        in_maps.append(
            {
                "xnT": xnT,
                "xiT": np.ascontiguousarray(xiT),
                "ycls": np.ascontiguousarray(ycl),
                "y_bcast": ybc,
                "y_own": yis,
            }
        )
    return in_maps


def finalize(per_core_outs, per_core_cls, y):
    yi = np.asarray(y).astype(np.int64)
    down = np.zeros(N, np.float64)
    top = np.zeros(N, np.float64)
    for core in range(NCORES):
        o = np.asarray(per_core_outs[core], dtype=np.float64)  # [P, 3*NJOBS]
        cl = np.asarray(per_core_cls[core], dtype=np.float64)  # [P, 4*CH]
        for jid, (rho, G, kind, half) in enumerate(job_table(core)):
            if kind not in ("full", "B"):
                continue
            rows = slice(rho * P, (rho + 1) * P)
            top[rows] += o[:, jid]
            down[rows] += o[:, NJOBS + 2 * jid]
            if kind == "full":
                down[rows] += o[:, NJOBS + 2 * jid + 1]
        for G in range(4):
            blk = cl[:, G * CH : (G + 1) * CH]                 # [128 c', 2048 j]
            cols = np.arange(G * CH, (G + 1) * CH)
            down[cols] += blk[100, :]
            top[cols] += blk[yi[cols], np.arange(CH)]
    return np.float32(np.mean(np.log(down) - np.log(top)))


def kernel(x, y):
    from concourse.bass_utils import run_bass_kernel_spmd

    nc = _get_program()
    in_maps = make_in_maps(x, y)
    res = run_bass_kernel_spmd(nc, in_maps, list(range(NCORES)))
    return finalize(
        [r["out"] for r in res.results],
        [r["cls_out"] for r in res.results],
        y,
    )
